# revision 31
# baseline (speedup 1.0000x reference)
"""2-layer GCN on 8 trn2 NeuronCores — single fused SPMD launch.

Full inputs in, full outputs out. Host sorts edges by dst and packs them
into groups of <=128 dst-nodes / <=2048 edges (16 tiles of 128). Each core
owns a contiguous run of groups (balanced by edge count). Per-tile
segment-sum is a TensorE matmul with an on-device-built one-hot*(norm)
selection matrix, accumulated in PSUM.

One launch does everything on device:
  A: S0_c = Xperm_c @ W0.T      (slot-sharded)      -> AllGather S0
  B: H_c  = relu(seg_sum(S0[src]*norm, dst))        -> AllGather H
  C: Z_c  = seg_sum(H[src]*norm, dst) @ W1.T        (stored transposed)

Unified slot layout: every node (including ones with no in-edges, which
land in otherwise-unused pad slots and correctly produce H=0) gets one
slot in the group-packed layout, and the host permutes X into slot order.
S0 and H then share the same all-gathered layout, so ONE u16 index per
edge serves both layers' gathers.

Host<->device tunnel traffic dominates wall time (~40-50 MB/s, ~0.08 s
fixed cost per array), so every stream is squeezed: X ships int8 with
per-feature-dim scales folded into W0.T, indices u16, slot ids int8,
norms bf16, on-device intermediates bf16, and the fused H/Z output is
int8 with per-row f32 scales bit-packed into 4 trailing byte columns of
the single output tensor. The PJRT executable is AOT-compiled at build
time (persistent jax compilation cache + neuron NEFF cache make this
fast on repeat runs); the timed section is transfer + execute + readback.
"""

import os
import re
import time

import numpy as np
from ml_dtypes import bfloat16

import jax

jax.config.update("jax_compilation_cache_dir",
                  os.path.expanduser("~/.jax_comp_cache"))
jax.config.update("jax_persistent_cache_min_entry_size_bytes", -1)
jax.config.update("jax_persistent_cache_min_compile_time_secs", 0)
# canonicalize source paths in HLO metadata so the compilation-cache key
# doesn't depend on the directory this file runs from
jax.config.update("jax_hlo_source_file_canonicalization_regex", ".*")

import jax.numpy as jnp
from jax.sharding import Mesh, NamedSharding, PartitionSpec
from jax.experimental.shard_map import shard_map

import concourse.bacc as bacc
import concourse.bass as bass
import concourse.bass2jax as b2j
import concourse.tile as tile
from concourse import mybir

P = 128
TPG = 16                 # tiles (of 128 edges) per group
EPG = P * TPG            # 2048 edge slots per group
NCORES = 8
N = 50000
D = 128
F32 = mybir.dt.float32
BF16 = mybir.dt.bfloat16
I32 = mybir.dt.int32
I8 = mybir.dt.int8
U16 = mybir.dt.uint16

LAST_TIMES = {}


def _pack_groups(dst_sorted):
    """Greedy pack sorted dst nodes into groups (<=P nodes, <=EPG edges).
    Returns list of (edge_start, edge_cnt, node_ids ndarray)."""
    nodes, counts = np.unique(dst_sorted, return_counts=True)
    groups = []
    i, e = 0, 0
    nn = len(nodes)
    while i < nn:
        es = e
        ns = i
        cnt_e = 0
        while i < nn and (i - ns) < P and cnt_e + counts[i] <= EPG:
            cnt_e += int(counts[i])
            i += 1
        assert i > ns, "single node exceeds group capacity"
        e += cnt_e
        groups.append((es, cnt_e, nodes[ns:i]))
    return groups


def _build_fused(G):
    """G groups of P node slots per core; S0/H share the slot layout."""
    SPC = G * P  # slots per core
    # disable_frame_to_traceback keeps python source locations out of the
    # emitted program, so the NEFF/XLA caches stay warm across file edits
    # and directory moves
    # constant debug info class-wide (before Bacc() so constructor-time
    # allocations are covered too): keeps file paths and line numbers out
    # of the BIR, so the serialized program (and every compile-cache key
    # derived from it) is identical regardless of where this file lives
    _dbg = mybir.OpDebugInfo(filename="kernel.py", lineno=0, kernel_name="k")
    bass.Bass.get_debug_info = lambda self: _dbg
    nc = bacc.Bacc(None, target_bir_lowering=False, num_swdge_queues=4,
                   num_devices=NCORES, disable_frame_to_traceback=True)
    # a few allocations capture debug info on the rust side, bypassing the
    # patch above; canonicalize those at serialization time so the program
    # bytes stay caller- and path-independent
    def _canon_json_bytes():
        b = bass.Bass.to_json_bytes(nc)
        b = re.sub(rb'"ant_traceback":"(?:[^"\\]|\\.)*"',
                   b'"ant_traceback":null', b)
        b = re.sub(rb'"filename":"[^"]*"', b'"filename":"kernel.py"', b)
        b = re.sub(rb'"lineno":\d+', b'"lineno":0', b)
        return b
    nc.to_json_bytes = _canon_json_bytes
    # X ships int8 in slot order, quantized per feature dim; the dequant
    # scales are folded into w0t rows on the host
    xt = nc.declare_dram_parameter("xt", [D, SPC], I8, isOutput=False)
    w01 = nc.declare_dram_parameter("w01", [2 * D, D], BF16, isOutput=False)
    # per edge slot: 2B u16 gather index + 1B slot id + 2B bf16 norm,
    # byte-packed into one tensor (cols 0:32 idx, 32:48 slot, 48:80 norm)
    edata = nc.declare_dram_parameter("edata", [G, P, 5 * TPG], I8, isOutput=False)
    # single fused output (H rows then Z.T rows), int8 with per-row f32
    # scales bit-packed into 4 trailing byte columns: halves the dominant
    # d2h stream vs bf16 at ~0.7% added error, in one contiguous transfer
    hz = nc.declare_dram_parameter("hz", [2 * SPC, D + 4], I8, isOutput=True)

    with tile.TileContext(nc) as tc:
        with (
            tc.tile_pool(name="dram", bufs=1, space="DRAM") as dram,
            tc.tile_pool(name="const", bufs=1) as cpool,
            tc.tile_pool(name="sbuf", bufs=4) as pool,
            tc.tile_pool(name="psum", bufs=2, space="PSUM") as psum,
            tc.tile_pool(name="psum2", bufs=2, space="PSUM") as psum2,
        ):
            s0_loc = dram.tile([SPC, D], BF16)
            s0_full = dram.tile([NCORES * SPC, D], BF16)
            h_loc = dram.tile([SPC, D], BF16)
            h_full = dram.tile([NCORES * SPC, D], BF16)

            iota_i = cpool.tile([P, P], dtype=I32)
            nc.gpsimd.iota(iota_i[:], pattern=[[1, P]], base=0,
                           channel_multiplier=0)
            iota_sb = cpool.tile([P, P], dtype=BF16)
            nc.vector.tensor_copy(iota_sb[:], iota_i[:])
            w0t_sb = cpool.tile([D, D], dtype=BF16)
            nc.sync.dma_start(out=w0t_sb[:], in_=w01[:D, :])
            w1t_sb = cpool.tile([D, D], dtype=BF16)
            nc.sync.dma_start(out=w1t_sb[:], in_=w01[D:, :])

            # ---- phase A: S0_c = Xperm_c @ W0.T (X arrives transposed) ----
            for t in range(G):
                xq_sb = pool.tile([P, P], dtype=I8, tag="xq")
                nc.sync.dma_start(out=xq_sb[:], in_=xt[:, t * P:(t + 1) * P])
                xt_sb = pool.tile([P, P], dtype=BF16, tag="xt")
                nc.vector.tensor_copy(xt_sb[:], xq_sb[:])
                s_ps = psum.tile([P, D], dtype=F32, tag="s")
                nc.tensor.matmul(out=s_ps[:], lhsT=xt_sb[:], rhs=w0t_sb[:],
                                 start=True, stop=True)
                s_sb = pool.tile([P, D], dtype=BF16, tag="s0")
                nc.vector.tensor_copy(s_sb[:], s_ps[:])
                nc.sync.dma_start(out=s0_loc[t * P:(t + 1) * P, :], in_=s_sb[:])

            nc.gpsimd.collective_compute(
                "AllGather", mybir.AluOpType.bypass,
                replica_groups=[list(range(NCORES))],
                ins=[s0_loc[:].opt()], outs=[s0_full[:].opt()],
            )

            # ---- phase B: H = relu(seg_sum(S0[src]*norm, dst)) ----
            for g in range(G):
                ed_sb = pool.tile([P, 5 * TPG], dtype=I8, tag="ed")
                nc.sync.dma_start(out=ed_sb[:], in_=edata[g])
                idx_sb = pool.tile([P, TPG], dtype=I32, tag="idx")
                nc.vector.tensor_copy(idx_sb[:], ed_sb[:, :2 * TPG].bitcast(U16))
                sl_sb = pool.tile([P, TPG], dtype=BF16, tag="sl")
                nc.vector.tensor_copy(sl_sb[:], ed_sb[:, 2 * TPG:3 * TPG])
                nrm_sb = pool.tile([P, TPG], dtype=F32, tag="nrm")
                nc.vector.tensor_copy(nrm_sb[:],
                                      ed_sb[:, 3 * TPG:].bitcast(BF16))
                acc_ps = psum.tile([P, D], dtype=F32, tag="acc")
                for t in range(TPG):
                    g_sb = pool.tile([P, D], dtype=BF16, tag="gat")
                    nc.gpsimd.indirect_dma_start(
                        out=g_sb[:], out_offset=None, in_=s0_full[:],
                        in_offset=bass.IndirectOffsetOnAxis(
                            ap=idx_sb[:, t:t + 1], axis=0),
                    )
                    sel = pool.tile([P, P], dtype=BF16, tag="sel")
                    nc.vector.tensor_tensor(
                        out=sel[:], in0=sl_sb[:, t:t + 1].to_broadcast([P, P])[:],
                        in1=iota_sb[:], op=mybir.AluOpType.is_equal,
                    )
                    pm = pool.tile([P, P], dtype=BF16, tag="pm")
                    nc.vector.tensor_scalar_mul(
                        pm[:], sel[:], nrm_sb[:, t:t + 1])
                    nc.tensor.matmul(out=acc_ps[:], lhsT=pm[:], rhs=g_sb[:],
                                     start=(t == 0), stop=(t == TPG - 1))
                h_sb = pool.tile([P, D], dtype=BF16, tag="h")
                nc.scalar.activation(h_sb[:], acc_ps[:],
                                     mybir.ActivationFunctionType.Relu)
                nc.sync.dma_start(out=h_loc[g * P:(g + 1) * P, :], in_=h_sb[:])
                # int8-quantize H rows (relu output >= 0, so max == absmax)
                m_sb = pool.tile([P, 1], dtype=F32, tag="m")
                nc.vector.reduce_max(m_sb[:], h_sb[:], axis=mybir.AxisListType.X)
                s_sb = pool.tile([P, 1], dtype=F32, tag="s")
                nc.scalar.activation(s_sb[:], m_sb[:],
                                     mybir.ActivationFunctionType.Copy,
                                     bias=1e-20, scale=1.0 / 127.0)
                qs_sb = pool.tile([P, 1], dtype=F32, tag="qs")
                nc.vector.reciprocal(qs_sb[:], s_sb[:])
                q_sb = pool.tile([P, D], dtype=I8, tag="q")
                nc.vector.tensor_scalar_mul(q_sb[:], h_sb[:], qs_sb[:, 0:1])
                nc.sync.dma_start(out=hz[g * P:(g + 1) * P, :D], in_=q_sb[:])
                nc.sync.dma_start(out=hz[g * P:(g + 1) * P, D:],
                                  in_=s_sb[:].bitcast(I8))

            nc.gpsimd.collective_compute(
                "AllGather", mybir.AluOpType.bypass,
                replica_groups=[list(range(NCORES))],
                ins=[h_loc[:].opt()], outs=[h_full[:].opt()],
            )

            # ---- phase C: Z = seg_sum(H[src]*norm, dst) @ W1.T ----
            # Accumulate transposed (accT = gathered.T @ pm) so the final
            # matmul zT = w1t.T @ accT needs no PE transpose. The hz Z-half
            # holds Z_g.T per group; the host transposes back.
            for g in range(G):
                ed_sb = pool.tile([P, 5 * TPG], dtype=I8, tag="ed")
                nc.sync.dma_start(out=ed_sb[:], in_=edata[g])
                idx_sb = pool.tile([P, TPG], dtype=I32, tag="idx")
                nc.vector.tensor_copy(idx_sb[:], ed_sb[:, :2 * TPG].bitcast(U16))
                sl_sb = pool.tile([P, TPG], dtype=BF16, tag="sl")
                nc.vector.tensor_copy(sl_sb[:], ed_sb[:, 2 * TPG:3 * TPG])
                nrm_sb = pool.tile([P, TPG], dtype=F32, tag="nrm")
                nc.vector.tensor_copy(nrm_sb[:],
                                      ed_sb[:, 3 * TPG:].bitcast(BF16))
                acc_ps = psum.tile([P, P], dtype=F32, tag="acc")
                for t in range(TPG):
                    g_sb = pool.tile([P, D], dtype=BF16, tag="gat")
                    nc.gpsimd.indirect_dma_start(
                        out=g_sb[:], out_offset=None, in_=h_full[:],
                        in_offset=bass.IndirectOffsetOnAxis(
                            ap=idx_sb[:, t:t + 1], axis=0),
                    )
                    sel = pool.tile([P, P], dtype=BF16, tag="sel")
                    nc.vector.tensor_tensor(
                        out=sel[:], in0=sl_sb[:, t:t + 1].to_broadcast([P, P])[:],
                        in1=iota_sb[:], op=mybir.AluOpType.is_equal,
                    )
                    pm = pool.tile([P, P], dtype=BF16, tag="pm")
                    nc.vector.tensor_scalar_mul(
                        pm[:], sel[:], nrm_sb[:, t:t + 1])
                    nc.tensor.matmul(out=acc_ps[:], lhsT=g_sb[:], rhs=pm[:],
                                     start=(t == 0), stop=(t == TPG - 1))
                at_sb = pool.tile([P, P], dtype=BF16, tag="aT")
                nc.vector.tensor_copy(at_sb[:], acc_ps[:])
                z_ps = psum2.tile([P, P], dtype=F32, tag="zT")
                nc.tensor.matmul(out=z_ps[:], lhsT=w1t_sb[:], rhs=at_sb[:],
                                 start=True, stop=True)
                # int8-quantize Z.T rows (per out-dim within the group)
                m_sb = pool.tile([P, 1], dtype=F32, tag="m")
                nc.vector.reduce_max(m_sb[:], z_ps[:], axis=mybir.AxisListType.X,
                                     apply_absolute_value=True)
                s_sb = pool.tile([P, 1], dtype=F32, tag="s")
                nc.scalar.activation(s_sb[:], m_sb[:],
                                     mybir.ActivationFunctionType.Copy,
                                     bias=1e-20, scale=1.0 / 127.0)
                qs_sb = pool.tile([P, 1], dtype=F32, tag="qs")
                nc.vector.reciprocal(qs_sb[:], s_sb[:])
                q_sb = pool.tile([P, P], dtype=I8, tag="q")
                nc.vector.tensor_scalar_mul(q_sb[:], z_ps[:], qs_sb[:, 0:1])
                nc.sync.dma_start(out=hz[(G + g) * P:(G + g + 1) * P, :D],
                                  in_=q_sb[:])
                nc.sync.dma_start(out=hz[(G + g) * P:(G + g + 1) * P, D:],
                                  in_=s_sb[:].bitcast(I8))
    nc.compile()
    return nc


def _prepare_exec(nc):
    """AOT-compile the SPMD executable (mirrors run_bass_via_pjrt, but with
    lowering/compilation split out so the timed section is exec-only), and
    materialize the donated zero output buffers directly on device."""
    b2j.install_neuronx_cc_hook()
    partition_name = nc.partition_id_tensor.name if nc.partition_id_tensor else None
    in_names, out_names, out_avals, zero_shapes = [], [], [], []
    for alloc in nc.m.functions[0].allocations:
        if not isinstance(alloc, mybir.MemoryLocationSet):
            continue
        name = alloc.memorylocations[0].name
        if alloc.kind == "ExternalInput":
            if name != partition_name:
                in_names.append(name)
        elif alloc.kind == "ExternalOutput":
            out_names.append(name)
            shape = tuple(alloc.tensor_shape)
            dtype = mybir.dt.np(alloc.dtype)
            out_avals.append(jax.core.ShapedArray(shape, dtype))
            zero_shapes.append((shape, dtype))
    n_params = len(in_names)
    n_outs = len(out_avals)
    in_names = in_names + out_names
    if partition_name is not None:
        in_names.append(partition_name)
    donate = tuple(range(n_params, n_params + n_outs))

    def _body(*args):
        operands = list(args)
        if partition_name is not None:
            operands.append(b2j.partition_id_tensor())
        outs = b2j._bass_exec_p.bind(
            *operands, out_avals=tuple(out_avals), in_names=tuple(in_names),
            out_names=tuple(out_names), lowering_input_output_aliases=(),
            sim_require_finite=True, sim_require_nnan=True, nc=nc)
        return tuple(outs)

    devices = jax.devices()[:NCORES]
    mesh = Mesh(np.asarray(devices), ("core",))
    spec = PartitionSpec("core")
    in_specs = (spec,) * (n_params + n_outs)
    out_specs = (spec,) * n_outs
    sharded = jax.jit(
        shard_map(_body, mesh=mesh, in_specs=in_specs, out_specs=out_specs,
                  check_rep=False),
        donate_argnums=donate, keep_unused=True)

    def g_struct(shape, dtype):
        return jax.ShapeDtypeStruct((NCORES * shape[0], *shape[1:]), dtype)

    in_structs = []
    # parameter avals in declaration order, via the module allocations again
    shapes_by_name = {}
    for alloc in nc.m.functions[0].allocations:
        if isinstance(alloc, mybir.MemoryLocationSet) and alloc.kind == "ExternalInput":
            shapes_by_name[alloc.memorylocations[0].name] = (
                tuple(alloc.tensor_shape), mybir.dt.np(alloc.dtype))
    for name in in_names[:n_params]:
        shp, dt = shapes_by_name[name]
        in_structs.append(g_struct(shp, dt))
    zero_structs = [g_struct(shp, dt) for shp, dt in zero_shapes]
    compiled = sharded.lower(*in_structs, *zero_structs).compile()

    sharding = NamedSharding(mesh, spec)
    return compiled, in_names[:n_params], out_names, out_avals, zero_shapes, sharding


_EXEC_CACHE = {}


def _get_exec(G):
    """Compiled executable per G; donated zero buffers are made fresh per
    call (donation consumes them) directly on device."""
    if G not in _EXEC_CACHE:
        _EXEC_CACHE[G] = _prepare_exec(_build_fused(G))
    compiled, in_names, out_names, out_avals, zero_shapes, sharding = _EXEC_CACHE[G]
    zeros_dev = [
        jax.jit(lambda s=shp, d=dt: jnp.zeros((NCORES * s[0], *s[1:]), d),
                out_shardings=sharding)()
        for shp, dt in zero_shapes]
    jax.block_until_ready(zeros_dev)
    return compiled, in_names, out_names, out_avals, zeros_dev, sharding


def kernel(X, W0, W1, norm, src, dst):
    t0 = time.perf_counter()
    X = np.asarray(X, dtype=np.float32)
    W0 = np.asarray(W0, dtype=np.float32)
    W1 = np.asarray(W1, dtype=np.float32)
    norm = np.asarray(norm, dtype=np.float32)
    src = np.asarray(src).astype(np.int64)
    dst = np.asarray(dst).astype(np.int64)
    E = src.shape[0]

    # ---- host preprocessing: sort by dst, pack groups, shard to cores ----
    order = np.argsort(dst, kind="stable")
    src_s = src[order].astype(np.int32)
    dst_s = dst[order]
    norm_s = norm[order]
    groups = _pack_groups(dst_s)
    cum = np.cumsum([g[1] for g in groups])
    core_of = np.minimum((NCORES * (cum - 1) // E).astype(np.int64), NCORES - 1)
    per_core = [[] for _ in range(NCORES)]
    for gi, g in enumerate(groups):
        per_core[int(core_of[gi])].append(g)
    # enough slots for every node, even if few nodes have in-edges
    G = max(max(len(lst) for lst in per_core), -(-N // (NCORES * P)))
    SPC = G * P

    # one slot per node in the group-packed layout; in-edge nodes keep their
    # group rank, the rest fill unused slots (their H rows compute to 0)
    slot_of_node = np.full(N, -1, dtype=np.int64)
    slot_arr = np.full((NCORES, G, P, TPG), -1, dtype=np.int8)
    sn_arr = np.zeros((NCORES, G, P, TPG), dtype=bfloat16)
    edge_pos = []  # per core: (g_i, p_i, t_i, edge range) for idx fill later
    asm_rows, asm_ids = [], []
    for c in range(NCORES):
        rows_l, ids_l = [], []
        for g_i, (es, ce, node_ids) in enumerate(per_core[c]):
            d_loc = np.searchsorted(node_ids, dst_s[es:es + ce]).astype(np.float32)
            j = np.arange(ce)
            t_i, p_i = j // P, j % P
            slot_arr[c, g_i, p_i, t_i] = d_loc.astype(np.int8)
            sn_arr[c, g_i, p_i, t_i] = norm_s[es:es + ce].astype(bfloat16)
            slot_of_node[node_ids] = c * SPC + g_i * P + np.arange(len(node_ids))
            rows_l.append(g_i * P + np.arange(len(node_ids)))
            ids_l.append(node_ids)
        asm_rows.append(np.concatenate(rows_l) if rows_l else np.zeros(0, np.int64))
        asm_ids.append(np.concatenate(ids_l) if ids_l else np.zeros(0, np.int64))
    free_mask = np.ones(NCORES * SPC, dtype=bool)
    assigned = slot_of_node[slot_of_node >= 0]
    free_mask[assigned] = False
    leftover = np.flatnonzero(slot_of_node < 0)
    if len(leftover):
        free = np.flatnonzero(free_mask)
        assert len(free) >= len(leftover), "not enough slots for all nodes"
        slot_of_node[leftover] = free[:len(leftover)]

    # one u16 gather index per edge slot, shared by both layers
    pos = slot_of_node[src_s].astype(np.uint16)
    idx_arr = np.zeros((NCORES, G, P, TPG), dtype=np.uint16)
    for c in range(NCORES):
        for g_i, (es, ce, node_ids) in enumerate(per_core[c]):
            j = np.arange(ce)
            idx_arr[c, g_i, j % P, j // P] = pos[es:es + ce]
    edata_arr = np.concatenate(
        [idx_arr.view(np.int8), slot_arr, sn_arr.view(np.int8)], axis=3)

    # quantize X per feature dim; fold the dequant scales into W0T rows so
    # the device never sees them. X is permuted into slot order.
    xsc = (np.abs(X).max(axis=0) / 127.0 + 1e-20).astype(np.float32)
    Xq = np.round(X / xsc).astype(np.int8)
    W0T = np.ascontiguousarray(W0.T * xsc[:, None]).astype(bfloat16)
    W1T = np.ascontiguousarray(W1.T).astype(bfloat16)
    W01 = np.concatenate([W0T, W1T])
    Xperm = np.zeros((NCORES * SPC, D), dtype=np.int8)
    Xperm[slot_of_node] = Xq
    XT = np.ascontiguousarray(
        Xperm.reshape(NCORES, SPC, D).transpose(0, 2, 1))
    LAST_TIMES["prep_s"] = time.perf_counter() - t0

    t1 = time.perf_counter()
    (compiled, in_names, out_names, out_avals, zeros_dev,
     sharding) = _get_exec(G)
    LAST_TIMES["build_s"] = time.perf_counter() - t1

    per_core_in = {
        "xt": XT,
        "w01": np.broadcast_to(W01, (NCORES, 2 * D, D)),
        "edata": edata_arr,
    }
    concat_in = [np.ascontiguousarray(per_core_in[name]).reshape(
        -1, *per_core_in[name].shape[2:]) for name in in_names]

    t1 = time.perf_counter()
    dev_in = jax.device_put(concat_in, [sharding] * len(concat_in))
    out_arrs = compiled(*dev_in, *zeros_dev)
    res = [np.asarray(a) for a in out_arrs]
    LAST_TIMES["run_fused_s"] = time.perf_counter() - t1

    hz_q = res[out_names.index("hz")].reshape(NCORES, 2 * SPC, D + 4)
    hsc = np.ascontiguousarray(hz_q[:, :, D:]).view(np.float32)
    # H: gather every node's row from the slot-ordered H halves
    h_all = (hz_q[:, :SPC, :D].astype(np.float32)
             * hsc[:, :SPC]).reshape(NCORES * SPC, D)
    H = np.ascontiguousarray(h_all[slot_of_node])
    Z = np.zeros((N, D), dtype=np.float32)
    for c in range(NCORES):
        zc = hz_q[c, SPC:, :D].astype(np.float32) * hsc[c, SPC:]
        zc = zc.reshape(G, P, P).transpose(0, 2, 1).reshape(SPC, P)
        Z[asm_ids[c]] = zc[asm_rows[c]]

    LAST_TIMES["total_s"] = time.perf_counter() - t0
    return (Z, H)


# revision 32
# speedup vs baseline: 1.0652x; 1.0652x over previous
"""2-layer GCN on 8 trn2 NeuronCores — single fused SPMD launch.

Full inputs in, full outputs out. Host sorts edges by dst and packs them
into groups of <=128 dst-nodes / <=2048 edges (16 tiles of 128). Each core
owns a contiguous run of groups (balanced by edge count). Per-tile
segment-sum is a TensorE matmul with an on-device-built one-hot*(norm)
selection matrix, accumulated in PSUM.

One launch does everything on device:
  A: S0_c = Xperm_c @ W0.T      (slot-sharded)      -> AllGather S0
  B: H_c  = relu(seg_sum(S0[src]*norm, dst))        -> AllGather H
  C: Z_c  = seg_sum(H[src]*norm, dst) @ W1.T        (stored transposed)

Unified slot layout: every node (including ones with no in-edges, which
land in otherwise-unused pad slots and correctly produce H=0) gets one
slot in the group-packed layout, and the host permutes X into slot order.
S0 and H then share the same all-gathered layout, so ONE u16 index per
edge serves both layers' gathers.

Host<->device tunnel traffic dominates wall time (~40-50 MB/s, ~0.08 s
fixed cost per array), so every stream is squeezed: X ships int8 with
per-feature-dim scales folded into W0.T, indices u16, slot ids int8,
norms bf16, on-device intermediates bf16, and the fused H/Z output is
int8 with per-row f32 scales bit-packed into 4 trailing byte columns of
the single output tensor. The PJRT executable is AOT-compiled at build
time (persistent jax compilation cache + neuron NEFF cache make this
fast on repeat runs); the timed section is transfer + execute + readback.
"""

import os
import re
import time

import numpy as np
from ml_dtypes import bfloat16

import jax

jax.config.update("jax_compilation_cache_dir",
                  os.path.expanduser("~/.jax_comp_cache"))
jax.config.update("jax_persistent_cache_min_entry_size_bytes", -1)
jax.config.update("jax_persistent_cache_min_compile_time_secs", 0)
# canonicalize source paths in HLO metadata so the compilation-cache key
# doesn't depend on the directory this file runs from
jax.config.update("jax_hlo_source_file_canonicalization_regex", ".*")

import jax.numpy as jnp
from jax.sharding import Mesh, NamedSharding, PartitionSpec
from jax.experimental.shard_map import shard_map

import concourse.bacc as bacc
import concourse.bass as bass
import concourse.bass2jax as b2j
import concourse.tile as tile
from concourse import mybir

P = 128
TPG = 16                 # tiles (of 128 edges) per group
EPG = P * TPG            # 2048 edge slots per group
NCORES = 8
N = 50000
D = 128
F32 = mybir.dt.float32
BF16 = mybir.dt.bfloat16
I32 = mybir.dt.int32
I8 = mybir.dt.int8
U16 = mybir.dt.uint16

LAST_TIMES = {}


def _pack_groups(dst_sorted):
    """Greedy pack sorted dst nodes into groups (<=P nodes, <=EPG edges).
    Returns list of (edge_start, edge_cnt, node_ids ndarray)."""
    nodes, counts = np.unique(dst_sorted, return_counts=True)
    groups = []
    i, e = 0, 0
    nn = len(nodes)
    while i < nn:
        es = e
        ns = i
        cnt_e = 0
        while i < nn and (i - ns) < P and cnt_e + counts[i] <= EPG:
            cnt_e += int(counts[i])
            i += 1
        assert i > ns, "single node exceeds group capacity"
        e += cnt_e
        groups.append((es, cnt_e, nodes[ns:i]))
    return groups


def _build_fused(G):
    """G groups of P node slots per core; S0/H share the slot layout."""
    SPC = G * P  # slots per core
    # disable_frame_to_traceback keeps python source locations out of the
    # emitted program, so the NEFF/XLA caches stay warm across file edits
    # and directory moves
    # constant debug info class-wide (before Bacc() so constructor-time
    # allocations are covered too): keeps file paths and line numbers out
    # of the BIR, so the serialized program (and every compile-cache key
    # derived from it) is identical regardless of where this file lives
    _dbg = mybir.OpDebugInfo(filename="kernel.py", lineno=0, kernel_name="k")
    bass.Bass.get_debug_info = lambda self: _dbg
    nc = bacc.Bacc(None, target_bir_lowering=False, num_swdge_queues=4,
                   num_devices=NCORES, disable_frame_to_traceback=True)
    # a few allocations capture debug info on the rust side, bypassing the
    # patch above; canonicalize those at serialization time so the program
    # bytes stay caller- and path-independent
    def _canon_json_bytes():
        b = bass.Bass.to_json_bytes(nc)
        b = re.sub(rb'"ant_traceback":"(?:[^"\\]|\\.)*"',
                   b'"ant_traceback":null', b)
        b = re.sub(rb'"filename":"[^"]*"', b'"filename":"kernel.py"', b)
        b = re.sub(rb'"lineno":\d+', b'"lineno":0', b)
        return b
    nc.to_json_bytes = _canon_json_bytes
    # X ships int8 in slot order, quantized per feature dim; the dequant
    # scales are folded into w0t rows on the host
    xt = nc.declare_dram_parameter("xt", [D, SPC], I8, isOutput=False)
    # weights ship sharded (32 rows per core) and are AllGathered on
    # device — replicating 0.5MB over the 40MB/s tunnel costs more than
    # one tiny collective
    w01s = nc.declare_dram_parameter("w01s", [2 * D // NCORES, D], BF16,
                                     isOutput=False)
    # per edge slot: 2B u16 gather index + 1B slot id + 2B bf16 norm,
    # byte-packed into one tensor (cols 0:32 idx, 32:48 slot, 48:80 norm)
    edata = nc.declare_dram_parameter("edata", [G, P, 5 * TPG], I8, isOutput=False)
    # single fused output (H rows then Z.T rows), int8 with per-row f32
    # scales bit-packed into 4 trailing byte columns: halves the dominant
    # d2h stream vs bf16 at ~0.7% added error, in one contiguous transfer
    hz = nc.declare_dram_parameter("hz", [2 * SPC, D + 4], I8, isOutput=True)

    with tile.TileContext(nc) as tc:
        with (
            tc.tile_pool(name="dram", bufs=1, space="DRAM") as dram,
            tc.tile_pool(name="const", bufs=1) as cpool,
            tc.tile_pool(name="sbuf", bufs=4) as pool,
            tc.tile_pool(name="psum", bufs=2, space="PSUM") as psum,
            tc.tile_pool(name="psum2", bufs=2, space="PSUM") as psum2,
        ):
            s0_loc = dram.tile([SPC, D], BF16)
            s0_full = dram.tile([NCORES * SPC, D], BF16)
            h_loc = dram.tile([SPC, D], BF16)
            h_full = dram.tile([NCORES * SPC, D], BF16)

            w01_loc = dram.tile([2 * D // NCORES, D], BF16)
            w01_full = dram.tile([2 * D, D], BF16)
            nc.sync.dma_start(out=w01_loc[:], in_=w01s[:])
            nc.gpsimd.collective_compute(
                "AllGather", mybir.AluOpType.bypass,
                replica_groups=[list(range(NCORES))],
                ins=[w01_loc[:].opt()], outs=[w01_full[:].opt()],
            )

            iota_i = cpool.tile([P, P], dtype=I32)
            nc.gpsimd.iota(iota_i[:], pattern=[[1, P]], base=0,
                           channel_multiplier=0)
            iota_sb = cpool.tile([P, P], dtype=BF16)
            nc.vector.tensor_copy(iota_sb[:], iota_i[:])
            w0t_sb = cpool.tile([D, D], dtype=BF16)
            nc.sync.dma_start(out=w0t_sb[:], in_=w01_full[:D, :])
            w1t_sb = cpool.tile([D, D], dtype=BF16)
            nc.sync.dma_start(out=w1t_sb[:], in_=w01_full[D:, :])

            # ---- phase A: S0_c = Xperm_c @ W0.T (X arrives transposed) ----
            for t in range(G):
                xq_sb = pool.tile([P, P], dtype=I8, tag="xq")
                nc.sync.dma_start(out=xq_sb[:], in_=xt[:, t * P:(t + 1) * P])
                xt_sb = pool.tile([P, P], dtype=BF16, tag="xt")
                nc.vector.tensor_copy(xt_sb[:], xq_sb[:])
                s_ps = psum.tile([P, D], dtype=F32, tag="s")
                nc.tensor.matmul(out=s_ps[:], lhsT=xt_sb[:], rhs=w0t_sb[:],
                                 start=True, stop=True)
                s_sb = pool.tile([P, D], dtype=BF16, tag="s0")
                nc.vector.tensor_copy(s_sb[:], s_ps[:])
                nc.sync.dma_start(out=s0_loc[t * P:(t + 1) * P, :], in_=s_sb[:])

            nc.gpsimd.collective_compute(
                "AllGather", mybir.AluOpType.bypass,
                replica_groups=[list(range(NCORES))],
                ins=[s0_loc[:].opt()], outs=[s0_full[:].opt()],
            )

            # ---- phase B: H = relu(seg_sum(S0[src]*norm, dst)) ----
            for g in range(G):
                ed_sb = pool.tile([P, 5 * TPG], dtype=I8, tag="ed")
                nc.sync.dma_start(out=ed_sb[:], in_=edata[g])
                idx_sb = pool.tile([P, TPG], dtype=I32, tag="idx")
                nc.vector.tensor_copy(idx_sb[:], ed_sb[:, :2 * TPG].bitcast(U16))
                sl_sb = pool.tile([P, TPG], dtype=BF16, tag="sl")
                nc.vector.tensor_copy(sl_sb[:], ed_sb[:, 2 * TPG:3 * TPG])
                nrm_sb = pool.tile([P, TPG], dtype=F32, tag="nrm")
                nc.vector.tensor_copy(nrm_sb[:],
                                      ed_sb[:, 3 * TPG:].bitcast(BF16))
                acc_ps = psum.tile([P, D], dtype=F32, tag="acc")
                for t in range(TPG):
                    g_sb = pool.tile([P, D], dtype=BF16, tag="gat")
                    nc.gpsimd.indirect_dma_start(
                        out=g_sb[:], out_offset=None, in_=s0_full[:],
                        in_offset=bass.IndirectOffsetOnAxis(
                            ap=idx_sb[:, t:t + 1], axis=0),
                    )
                    sel = pool.tile([P, P], dtype=BF16, tag="sel")
                    nc.vector.tensor_tensor(
                        out=sel[:], in0=sl_sb[:, t:t + 1].to_broadcast([P, P])[:],
                        in1=iota_sb[:], op=mybir.AluOpType.is_equal,
                    )
                    pm = pool.tile([P, P], dtype=BF16, tag="pm")
                    nc.vector.tensor_scalar_mul(
                        pm[:], sel[:], nrm_sb[:, t:t + 1])
                    nc.tensor.matmul(out=acc_ps[:], lhsT=pm[:], rhs=g_sb[:],
                                     start=(t == 0), stop=(t == TPG - 1))
                h_sb = pool.tile([P, D], dtype=BF16, tag="h")
                nc.scalar.activation(h_sb[:], acc_ps[:],
                                     mybir.ActivationFunctionType.Relu)
                nc.sync.dma_start(out=h_loc[g * P:(g + 1) * P, :], in_=h_sb[:])
                # int8-quantize H rows (relu output >= 0, so max == absmax)
                m_sb = pool.tile([P, 1], dtype=F32, tag="m")
                nc.vector.reduce_max(m_sb[:], h_sb[:], axis=mybir.AxisListType.X)
                s_sb = pool.tile([P, 1], dtype=F32, tag="s")
                nc.scalar.activation(s_sb[:], m_sb[:],
                                     mybir.ActivationFunctionType.Copy,
                                     bias=1e-20, scale=1.0 / 127.0)
                qs_sb = pool.tile([P, 1], dtype=F32, tag="qs")
                nc.vector.reciprocal(qs_sb[:], s_sb[:])
                q_sb = pool.tile([P, D], dtype=I8, tag="q")
                nc.vector.tensor_scalar_mul(q_sb[:], h_sb[:], qs_sb[:, 0:1])
                nc.sync.dma_start(out=hz[g * P:(g + 1) * P, :D], in_=q_sb[:])
                nc.sync.dma_start(out=hz[g * P:(g + 1) * P, D:],
                                  in_=s_sb[:].bitcast(I8))

            nc.gpsimd.collective_compute(
                "AllGather", mybir.AluOpType.bypass,
                replica_groups=[list(range(NCORES))],
                ins=[h_loc[:].opt()], outs=[h_full[:].opt()],
            )

            # ---- phase C: Z = seg_sum(H[src]*norm, dst) @ W1.T ----
            # Accumulate transposed (accT = gathered.T @ pm) so the final
            # matmul zT = w1t.T @ accT needs no PE transpose. The hz Z-half
            # holds Z_g.T per group; the host transposes back.
            for g in range(G):
                ed_sb = pool.tile([P, 5 * TPG], dtype=I8, tag="ed")
                nc.sync.dma_start(out=ed_sb[:], in_=edata[g])
                idx_sb = pool.tile([P, TPG], dtype=I32, tag="idx")
                nc.vector.tensor_copy(idx_sb[:], ed_sb[:, :2 * TPG].bitcast(U16))
                sl_sb = pool.tile([P, TPG], dtype=BF16, tag="sl")
                nc.vector.tensor_copy(sl_sb[:], ed_sb[:, 2 * TPG:3 * TPG])
                nrm_sb = pool.tile([P, TPG], dtype=F32, tag="nrm")
                nc.vector.tensor_copy(nrm_sb[:],
                                      ed_sb[:, 3 * TPG:].bitcast(BF16))
                acc_ps = psum.tile([P, P], dtype=F32, tag="acc")
                for t in range(TPG):
                    g_sb = pool.tile([P, D], dtype=BF16, tag="gat")
                    nc.gpsimd.indirect_dma_start(
                        out=g_sb[:], out_offset=None, in_=h_full[:],
                        in_offset=bass.IndirectOffsetOnAxis(
                            ap=idx_sb[:, t:t + 1], axis=0),
                    )
                    sel = pool.tile([P, P], dtype=BF16, tag="sel")
                    nc.vector.tensor_tensor(
                        out=sel[:], in0=sl_sb[:, t:t + 1].to_broadcast([P, P])[:],
                        in1=iota_sb[:], op=mybir.AluOpType.is_equal,
                    )
                    pm = pool.tile([P, P], dtype=BF16, tag="pm")
                    nc.vector.tensor_scalar_mul(
                        pm[:], sel[:], nrm_sb[:, t:t + 1])
                    nc.tensor.matmul(out=acc_ps[:], lhsT=g_sb[:], rhs=pm[:],
                                     start=(t == 0), stop=(t == TPG - 1))
                at_sb = pool.tile([P, P], dtype=BF16, tag="aT")
                nc.vector.tensor_copy(at_sb[:], acc_ps[:])
                z_ps = psum2.tile([P, P], dtype=F32, tag="zT")
                nc.tensor.matmul(out=z_ps[:], lhsT=w1t_sb[:], rhs=at_sb[:],
                                 start=True, stop=True)
                # int8-quantize Z.T rows (per out-dim within the group)
                m_sb = pool.tile([P, 1], dtype=F32, tag="m")
                nc.vector.reduce_max(m_sb[:], z_ps[:], axis=mybir.AxisListType.X,
                                     apply_absolute_value=True)
                s_sb = pool.tile([P, 1], dtype=F32, tag="s")
                nc.scalar.activation(s_sb[:], m_sb[:],
                                     mybir.ActivationFunctionType.Copy,
                                     bias=1e-20, scale=1.0 / 127.0)
                qs_sb = pool.tile([P, 1], dtype=F32, tag="qs")
                nc.vector.reciprocal(qs_sb[:], s_sb[:])
                q_sb = pool.tile([P, P], dtype=I8, tag="q")
                nc.vector.tensor_scalar_mul(q_sb[:], z_ps[:], qs_sb[:, 0:1])
                nc.sync.dma_start(out=hz[(G + g) * P:(G + g + 1) * P, :D],
                                  in_=q_sb[:])
                nc.sync.dma_start(out=hz[(G + g) * P:(G + g + 1) * P, D:],
                                  in_=s_sb[:].bitcast(I8))
    nc.compile()
    return nc


def _prepare_exec(nc):
    """AOT-compile the SPMD executable (mirrors run_bass_via_pjrt, but with
    lowering/compilation split out so the timed section is exec-only), and
    materialize the donated zero output buffers directly on device."""
    b2j.install_neuronx_cc_hook()
    partition_name = nc.partition_id_tensor.name if nc.partition_id_tensor else None
    in_names, out_names, out_avals, zero_shapes = [], [], [], []
    for alloc in nc.m.functions[0].allocations:
        if not isinstance(alloc, mybir.MemoryLocationSet):
            continue
        name = alloc.memorylocations[0].name
        if alloc.kind == "ExternalInput":
            if name != partition_name:
                in_names.append(name)
        elif alloc.kind == "ExternalOutput":
            out_names.append(name)
            shape = tuple(alloc.tensor_shape)
            dtype = mybir.dt.np(alloc.dtype)
            out_avals.append(jax.core.ShapedArray(shape, dtype))
            zero_shapes.append((shape, dtype))
    n_params = len(in_names)
    n_outs = len(out_avals)
    in_names = in_names + out_names
    if partition_name is not None:
        in_names.append(partition_name)
    donate = tuple(range(n_params, n_params + n_outs))

    def _body(*args):
        operands = list(args)
        if partition_name is not None:
            operands.append(b2j.partition_id_tensor())
        outs = b2j._bass_exec_p.bind(
            *operands, out_avals=tuple(out_avals), in_names=tuple(in_names),
            out_names=tuple(out_names), lowering_input_output_aliases=(),
            sim_require_finite=True, sim_require_nnan=True, nc=nc)
        return tuple(outs)

    devices = jax.devices()[:NCORES]
    mesh = Mesh(np.asarray(devices), ("core",))
    spec = PartitionSpec("core")
    in_specs = (spec,) * (n_params + n_outs)
    out_specs = (spec,) * n_outs
    sharded = jax.jit(
        shard_map(_body, mesh=mesh, in_specs=in_specs, out_specs=out_specs,
                  check_rep=False),
        donate_argnums=donate, keep_unused=True)

    def g_struct(shape, dtype):
        return jax.ShapeDtypeStruct((NCORES * shape[0], *shape[1:]), dtype)

    in_structs = []
    # parameter avals in declaration order, via the module allocations again
    shapes_by_name = {}
    for alloc in nc.m.functions[0].allocations:
        if isinstance(alloc, mybir.MemoryLocationSet) and alloc.kind == "ExternalInput":
            shapes_by_name[alloc.memorylocations[0].name] = (
                tuple(alloc.tensor_shape), mybir.dt.np(alloc.dtype))
    for name in in_names[:n_params]:
        shp, dt = shapes_by_name[name]
        in_structs.append(g_struct(shp, dt))
    zero_structs = [g_struct(shp, dt) for shp, dt in zero_shapes]
    compiled = sharded.lower(*in_structs, *zero_structs).compile()

    sharding = NamedSharding(mesh, spec)
    return compiled, in_names[:n_params], out_names, out_avals, zero_shapes, sharding


_EXEC_CACHE = {}


def _get_exec(G):
    """Compiled executable per G; donated zero buffers are made fresh per
    call (donation consumes them) directly on device."""
    if G not in _EXEC_CACHE:
        _EXEC_CACHE[G] = _prepare_exec(_build_fused(G))
    compiled, in_names, out_names, out_avals, zero_shapes, sharding = _EXEC_CACHE[G]
    zeros_dev = [
        jax.jit(lambda s=shp, d=dt: jnp.zeros((NCORES * s[0], *s[1:]), d),
                out_shardings=sharding)()
        for shp, dt in zero_shapes]
    jax.block_until_ready(zeros_dev)
    return compiled, in_names, out_names, out_avals, zeros_dev, sharding


def kernel(X, W0, W1, norm, src, dst):
    t0 = time.perf_counter()
    X = np.asarray(X, dtype=np.float32)
    W0 = np.asarray(W0, dtype=np.float32)
    W1 = np.asarray(W1, dtype=np.float32)
    norm = np.asarray(norm, dtype=np.float32)
    src = np.asarray(src).astype(np.int64)
    dst = np.asarray(dst).astype(np.int64)
    E = src.shape[0]

    # ---- host preprocessing: sort by dst, pack groups, shard to cores ----
    order = np.argsort(dst, kind="stable")
    src_s = src[order].astype(np.int32)
    dst_s = dst[order]
    norm_s = norm[order]
    groups = _pack_groups(dst_s)
    cum = np.cumsum([g[1] for g in groups])
    core_of = np.minimum((NCORES * (cum - 1) // E).astype(np.int64), NCORES - 1)
    per_core = [[] for _ in range(NCORES)]
    for gi, g in enumerate(groups):
        per_core[int(core_of[gi])].append(g)
    # enough slots for every node, even if few nodes have in-edges
    G = max(max(len(lst) for lst in per_core), -(-N // (NCORES * P)))
    SPC = G * P

    # one slot per node in the group-packed layout; in-edge nodes keep their
    # group rank, the rest fill unused slots (their H rows compute to 0)
    slot_of_node = np.full(N, -1, dtype=np.int64)
    slot_arr = np.full((NCORES, G, P, TPG), -1, dtype=np.int8)
    sn_arr = np.zeros((NCORES, G, P, TPG), dtype=bfloat16)
    edge_pos = []  # per core: (g_i, p_i, t_i, edge range) for idx fill later
    asm_rows, asm_ids = [], []
    for c in range(NCORES):
        rows_l, ids_l = [], []
        for g_i, (es, ce, node_ids) in enumerate(per_core[c]):
            d_loc = np.searchsorted(node_ids, dst_s[es:es + ce]).astype(np.float32)
            j = np.arange(ce)
            t_i, p_i = j // P, j % P
            slot_arr[c, g_i, p_i, t_i] = d_loc.astype(np.int8)
            sn_arr[c, g_i, p_i, t_i] = norm_s[es:es + ce].astype(bfloat16)
            slot_of_node[node_ids] = c * SPC + g_i * P + np.arange(len(node_ids))
            rows_l.append(g_i * P + np.arange(len(node_ids)))
            ids_l.append(node_ids)
        asm_rows.append(np.concatenate(rows_l) if rows_l else np.zeros(0, np.int64))
        asm_ids.append(np.concatenate(ids_l) if ids_l else np.zeros(0, np.int64))
    free_mask = np.ones(NCORES * SPC, dtype=bool)
    assigned = slot_of_node[slot_of_node >= 0]
    free_mask[assigned] = False
    leftover = np.flatnonzero(slot_of_node < 0)
    if len(leftover):
        free = np.flatnonzero(free_mask)
        assert len(free) >= len(leftover), "not enough slots for all nodes"
        slot_of_node[leftover] = free[:len(leftover)]

    # one u16 gather index per edge slot, shared by both layers
    pos = slot_of_node[src_s].astype(np.uint16)
    idx_arr = np.zeros((NCORES, G, P, TPG), dtype=np.uint16)
    for c in range(NCORES):
        for g_i, (es, ce, node_ids) in enumerate(per_core[c]):
            j = np.arange(ce)
            idx_arr[c, g_i, j % P, j // P] = pos[es:es + ce]
    edata_arr = np.concatenate(
        [idx_arr.view(np.int8), slot_arr, sn_arr.view(np.int8)], axis=3)

    # quantize X per feature dim; fold the dequant scales into W0T rows so
    # the device never sees them. X is permuted into slot order.
    xsc = (np.abs(X).max(axis=0) / 127.0 + 1e-20).astype(np.float32)
    Xq = np.round(X / xsc).astype(np.int8)
    W0T = np.ascontiguousarray(W0.T * xsc[:, None]).astype(bfloat16)
    W1T = np.ascontiguousarray(W1.T).astype(bfloat16)
    W01 = np.concatenate([W0T, W1T])
    Xperm = np.zeros((NCORES * SPC, D), dtype=np.int8)
    Xperm[slot_of_node] = Xq
    XT = np.ascontiguousarray(
        Xperm.reshape(NCORES, SPC, D).transpose(0, 2, 1))
    LAST_TIMES["prep_s"] = time.perf_counter() - t0

    t1 = time.perf_counter()
    (compiled, in_names, out_names, out_avals, zeros_dev,
     sharding) = _get_exec(G)
    LAST_TIMES["build_s"] = time.perf_counter() - t1

    per_core_in = {
        "xt": XT,
        "w01s": W01.reshape(NCORES, 2 * D // NCORES, D),
        "edata": edata_arr,
    }
    concat_in = [np.ascontiguousarray(per_core_in[name]).reshape(
        -1, *per_core_in[name].shape[2:]) for name in in_names]

    t1 = time.perf_counter()
    dev_in = jax.device_put(concat_in, [sharding] * len(concat_in))
    out_arrs = compiled(*dev_in, *zeros_dev)
    res = [np.asarray(a) for a in out_arrs]
    LAST_TIMES["run_fused_s"] = time.perf_counter() - t1

    hz_q = res[out_names.index("hz")].reshape(NCORES, 2 * SPC, D + 4)
    hsc = np.ascontiguousarray(hz_q[:, :, D:]).view(np.float32)
    # H: gather every node's row from the slot-ordered H halves
    h_all = (hz_q[:, :SPC, :D].astype(np.float32)
             * hsc[:, :SPC]).reshape(NCORES * SPC, D)
    H = np.ascontiguousarray(h_all[slot_of_node])
    Z = np.zeros((N, D), dtype=np.float32)
    for c in range(NCORES):
        zc = hz_q[c, SPC:, :D].astype(np.float32) * hsc[c, SPC:]
        zc = zc.reshape(G, P, P).transpose(0, 2, 1).reshape(SPC, P)
        Z[asm_ids[c]] = zc[asm_rows[c]]

    LAST_TIMES["total_s"] = time.perf_counter() - t0
    return (Z, H)
